# revision 1
# baseline (speedup 1.0000x reference)
"""Trainium2 Bass kernel for nn_Decoder_15187004358874.

Decoder transformer: action-encoder + 4 blocks of (causal self-attn,
causal cross-attn vs obs_rep, GELU MLP) + head.  B=256, N=64, D=512,
H=8, A=64.  Data-parallel over batch across 8 NeuronCores (32 seqs per
core); per-core kernel computes the whole network on its shard.

Layout strategy per core:
  - activations live token-major [128 tokens, 512] for LayerNorm
    (bn_stats) and residuals; feature-major f32r chunks (via PE
    transposes) feed all matmuls.
  - all matmuls run in float32r (full PE speed at free>=256, ~1.6e-4
    relative error), weights DMA'd straight into f32r SBUF tiles.
  - attention: block-diag q stationary [128,128] (head pair) x k moving
    -> scores [128(q x 2 heads), 64(k)]; softmax along free dim; PE
    transpose of normalized probs.  v is produced as full [128-token,
    512] tiles (M=128 matmuls), with the odd-sequence rows DMA-shifted
    down to partitions 0:64 so the fused AV matmuls (one N=128 MM per
    sequence, diagonal-block outputs) stay in base-0 geometry (f32r
    matmuls require output base partition 0, and offset-partition PE
    tricks proved fragile on hardware).
"""

import sys

sys.path.insert(0, "/opt/trn_rl_repo")

import numpy as np

import concourse.bass as bass
import concourse.mybir as mybir
import concourse.tile as tile
from concourse import bacc
from concourse.bass_utils import run_bass_kernel_spmd

F32 = mybir.dt.float32
F32R = mybir.dt.float32r
BF16 = mybir.dt.bfloat16
AF = mybir.ActivationFunctionType

B, N, D, H, A, OBS, NB = 256, 64, 512, 8, 64, 128, 4
HD = D // H          # 64
KC = D // 128        # 4 feature chunks
HP = H // 2          # 4 head pairs
EPS = 1e-5
N_CORES = 8
SEQS_PER_CORE = B // N_CORES   # 32


def build_program(n_seqs=SEQS_PER_CORE, n_blocks=NB, n_chunks=2,
                  use_bp=False, use_b2=False, use_bh1=False, use_bh2=False,
                  use_qk_bias=False, ln_affine=False, att_bf16=True):
    """Build the per-core Bass program.  Returns the compiled Bacc."""
    assert n_seqs % (4 * n_chunks) == 0
    chunk_seqs = n_seqs // n_chunks          # seqs per chunk
    n_groups = chunk_seqs // 4               # groups of 4 seqs
    c_tiles = chunk_seqs * N // 128          # 128-token tiles per chunk
    CT = chunk_seqs * N                      # tokens per chunk

    ATT_DT = BF16 if att_bf16 else F32R
    nc = bacc.Bacc("TRN2", target_bir_lowering=False, debug=False)

    # ---------------- DRAM parameters ----------------
    act_d = nc.dram_tensor("action", [n_seqs, N, A + 1], F32, kind="ExternalInput").ap()
    obs_d = nc.dram_tensor("obs_rep", [n_seqs, N, D], F32, kind="ExternalInput").ap()
    w_ae_d = nc.dram_tensor("w_ae", [A + 1, D], F32R, kind="ExternalInput").ap()
    wq_d = nc.dram_tensor("blk_wq", [NB, 2, D, D], F32R, kind="ExternalInput").ap()
    wk_d = nc.dram_tensor("blk_wk", [NB, 2, D, D], F32R, kind="ExternalInput").ap()
    wv_d = nc.dram_tensor("blk_wv", [NB, 2, D, D], F32R, kind="ExternalInput").ap()
    wp_d = nc.dram_tensor("blk_wp", [NB, 2, D, D], F32R, kind="ExternalInput").ap()
    bq_d = nc.dram_tensor("blk_bq", [NB, 2, D], F32, kind="ExternalInput").ap()
    bk_d = nc.dram_tensor("blk_bk", [NB, 2, D], F32, kind="ExternalInput").ap()
    bp_d = nc.dram_tensor("blk_bp", [NB, 2, D], F32, kind="ExternalInput").ap()
    w1_d = nc.dram_tensor("blk_w1", [NB, D, D], F32R, kind="ExternalInput").ap()
    b1_d = nc.dram_tensor("blk_b1", [NB, D], F32, kind="ExternalInput").ap()
    w2_d = nc.dram_tensor("blk_w2", [NB, D, D], F32R, kind="ExternalInput").ap()
    b2_d = nc.dram_tensor("blk_b2", [NB, D], F32, kind="ExternalInput").ap()
    wh1_d = nc.dram_tensor("wh1", [D, D], F32R, kind="ExternalInput").ap()
    bh1_d = nc.dram_tensor("bh1", [D], F32, kind="ExternalInput").ap()
    wh2_d = nc.dram_tensor("wh2", [D, A], F32R, kind="ExternalInput").ap()
    bh2_d = nc.dram_tensor("bh2", [A], F32, kind="ExternalInput").ap()
    lng_d = nc.dram_tensor("blk_ln_g", [NB, 3, D], F32, kind="ExternalInput").ap()
    lnb_d = nc.dram_tensor("blk_ln_b", [NB, 3, D], F32, kind="ExternalInput").ap()
    ln0g_d = nc.dram_tensor("ln0_g", [D], F32, kind="ExternalInput").ap()
    ln0b_d = nc.dram_tensor("ln0_b", [D], F32, kind="ExternalInput").ap()
    lnhg_d = nc.dram_tensor("lnh_g", [D], F32, kind="ExternalInput").ap()
    lnhb_d = nc.dram_tensor("lnh_b", [D], F32, kind="ExternalInput").ap()
    mask_d = nc.dram_tensor("const_mask", [128, 4 * N], F32, kind="ExternalInput").ap()
    ident_d = nc.dram_tensor("const_identity", [128, 128], F32, kind="ExternalInput").ap()
    out_d = nc.dram_tensor("logit", [n_seqs, N, A], F32, kind="ExternalOutput").ap()

    act_flat = act_d.rearrange("s n d -> (s n) d")
    obs_flat = obs_d.rearrange("s n d -> (s n) d")
    out_flat = out_d.rearrange("s n d -> (s n) d")

    with tile.TileContext(nc) as tc:
        _pools = []

        def _pool(**kw):
            p = tc.alloc_tile_pool(**kw)
            _pools.append(p)
            return p

        cst = _pool(name="cst", bufs=1)
        wp_pool = _pool(name="wp", bufs=4)
        res_pool = _pool(name="res", bufs=2)
        big_pool = _pool(name="big", bufs=1)
        grp_pool = _pool(name="grp", bufs=2)
        ln_pool = _pool(name="ln", bufs=3)
        lnp_big = _pool(name="lnb", bufs=10)
        sm_pool = _pool(name="sm", bufs=8)
        att_pool = _pool(name="att", bufs=3)
        pp_tr = _pool(name="pptr", bufs=2, space="PSUM")
        pp_lin = _pool(name="ppl", bufs=3, space="PSUM")
        pp_att = _pool(name="ppa", bufs=2, space="PSUM")
        pp_y = _pool(name="ppy", bufs=1, space="PSUM")

        # ---------------- constants ----------------
        identity = cst.tile([128, 128], F32)
        nc.sync.dma_start(out=identity, in_=ident_d)
        identity_r = cst.tile([128, 128], F32R)
        nc.vector.tensor_copy(out=identity_r, in_=identity)
        identity_a = cst.tile([128, 128], ATT_DT, name="identity_a")
        nc.vector.tensor_copy(out=identity_a, in_=identity)
        mask = cst.tile([128, 4 * N], F32)
        nc.sync.dma_start(out=mask, in_=mask_d)
        eps_t = cst.tile([128, 1], F32)
        nc.vector.memset(eps_t, EPS)

        def load_bias(vec_ap, name):
            t = cst.tile([128, KC], F32, name=name)
            nc.sync.dma_start(out=t, in_=vec_ap.rearrange("(c p) -> p c", p=128))
            return t

        bq_t = [[load_bias(bq_d[i, s], f"bq_{i}_{s}") for s in range(2)]
                for i in range(n_blocks)]
        bk_t = [[load_bias(bk_d[i, s], f"bk_{i}_{s}") for s in range(2)]
                for i in range(n_blocks)]
        b1_t = [load_bias(b1_d[i], f"b1_{i}") for i in range(n_blocks)]

        def bcast_tile(vec_ap, width, name):
            """Broadcast a [width] dram vector across 128 partitions."""
            t = cst.tile([128, width], F32, name=name)
            src = bass.AP(tensor=vec_ap.tensor, offset=vec_ap.offset,
                          ap=[[0, 128]] + vec_ap.ap)
            nc.sync.dma_start(out=t, in_=src)
            return t

        bp_bc = b2_bc = None
        if use_bp:
            bp_bc = [[bcast_tile(bp_d[i, s], D, f"bpb_{i}_{s}") for s in range(2)]
                     for i in range(n_blocks)]
        if use_b2:
            b2_bc = [bcast_tile(b2_d[i], D, f"b2b_{i}") for i in range(n_blocks)]
        bh1_bc = bcast_tile(bh1_d, D, "bh1b") if use_bh1 else None
        bh2_bc = bcast_tile(bh2_d, A, "bh2b") if use_bh2 else None
        ln_bc = None
        if ln_affine:
            ln_bc = {}
            for i in range(n_blocks):
                for j in range(3):
                    ln_bc[(i, j)] = (bcast_tile(lng_d[i, j], D, f"lng_{i}_{j}"),
                                     bcast_tile(lnb_d[i, j], D, f"lnb_{i}_{j}"))
            ln_bc["ln0"] = (bcast_tile(ln0g_d, D, "ln0g"), bcast_tile(ln0b_d, D, "ln0b"))
            ln_bc["lnh"] = (bcast_tile(lnhg_d, D, "lnhg"), bcast_tile(lnhb_d, D, "lnhb"))

        # persistent block-diag q tiles (off-diag stays zero forever)
        qbd_tiles = []
        for qi in range(4):
            qbt = cst.tile([128, 4, 128], ATT_DT, name=f"qbd{qi}")
            if att_bf16:
                nc.vector.memset(qbt, 0.0)
            else:
                nc.vector.memset(qbt.bitcast(F32), 0.0)
            qbd_tiles.append(qbt)

        # encoder + head weights (persistent)
        w_ae_t = cst.tile([A + 1, D], F32R)
        nc.sync.dma_start(out=w_ae_t, in_=w_ae_d)

        wh2_t = cst.tile([128, KC, A], F32R)
        nc.sync.dma_start(out=wh2_t, in_=wh2_d.rearrange("(c p) m -> p c m", p=128))

        # ---------------- helpers ----------------
        def load_w(dram_slice, name):
            t = wp_pool.tile([128, KC, D], F32R, tag="w", name=name)
            nc.sync.dma_start(out=t, in_=dram_slice.rearrange("(c p) m -> p c m", p=128))
            return t

        class LNBatch:
            """Collect per-tile bn stats; one batched sqrt/recip at flush
            (keeps the ACT engine inside one activation-table set)."""

            def __init__(self):
                self.mvb = sm_pool.tile([128, 8, 2], F32, name="mvb", tag="mvb")
                self.items = []

            def add(self, xpre, affine_key, target_fn, post=None):
                st = sm_pool.tile([128, 6], F32, name="st")
                nc.vector.bn_stats(out=st, in_=xpre)
                i = len(self.items)
                nc.vector.bn_aggr(out=self.mvb[:, i, :], in_=st)
                self.items.append((xpre, affine_key, target_fn, post))
                if i == 7:
                    self.flush()

            def flush(self):
                n = len(self.items)
                if n == 0:
                    return
                sd = sm_pool.tile([128, 8], F32, name="sd2", tag="sd2")
                nc.scalar.activation(out=sd[:, 0:n], in_=self.mvb[:, 0:n, 1],
                                     func=AF.Sqrt, bias=eps_t, scale=1.0)
                rstd = sm_pool.tile([128, 8], F32, name="rstd2", tag="rstd2")
                nc.vector.reciprocal(out=rstd[:, 0:n], in_=sd[:, 0:n])
                nmr = sm_pool.tile([128, 8], F32, name="nmr2", tag="nmr2")
                nc.vector.tensor_scalar(out=nmr[:, 0:n], in0=self.mvb[:, 0:n, 0],
                                        scalar1=-1.0, scalar2=None,
                                        op0=mybir.AluOpType.mult)
                nc.vector.tensor_mul(nmr[:, 0:n], nmr[:, 0:n], rstd[:, 0:n])
                for i, (xpre, key, target_fn, post) in enumerate(self.items):
                    tgt = target_fn()
                    if i % 2 == 0:
                        nc.vector.tensor_scalar(out=tgt, in0=xpre,
                                                scalar1=rstd[:, i:i + 1],
                                                scalar2=nmr[:, i:i + 1],
                                                op0=mybir.AluOpType.mult,
                                                op1=mybir.AluOpType.add)
                    else:
                        nc.scalar.activation(out=tgt, in_=xpre, func=AF.Identity,
                                             bias=nmr[:, i:i + 1],
                                             scale=rstd[:, i:i + 1])
                    if ln_affine and key is not None:
                        g_bc, b_bc = ln_bc[key]
                        nc.vector.tensor_mul(tgt, tgt, g_bc)
                        nc.vector.tensor_add(tgt, tgt, b_bc)
                    if post is not None:
                        post(tgt)
                self.items = []
                self.mvb = sm_pool.tile([128, 8, 2], F32, name="mvb", tag="mvb")

        def transpose_tm_tile(src_tile, dst_fm, dst_cols):
            """src [128 tok, D] f32 -> dst_fm[:, kc, dst_cols] f32r via PE."""
            ptr = pp_tr.tile([128, KC, 128], F32, tag="ptr", name="ptr")
            for kc_i in range(KC):
                nc.tensor.transpose(ptr[:, kc_i, :], src_tile[:, kc_i * 128:(kc_i + 1) * 128],
                                    identity)
            nc.any.tensor_copy(out=dst_fm[:, :, dst_cols], in_=ptr)

        # ================= main program =================
        for ci in range(n_chunks):
            tok0 = ci * CT  # first token of chunk

            # ---- residual tiles for this chunk ----
            x_res = res_pool.tile([128, c_tiles, D], F32, tag="res", name="x_res")
            obs_tm = big_pool.tile([128, c_tiles, D], F32, tag="obs", name="obs_tm")

            # ---- encoder ----
            act_fm = big_pool.tile([A + 1, CT], F32R, tag="actfm", name="act_fm")
            for t in range(c_tiles):
                at = ln_pool.tile([128, A + 1], F32, tag="at", name="at")
                nc.sync.dma_start(out=at, in_=act_flat[tok0 + t * 128: tok0 + (t + 1) * 128, :])
                pt = pp_tr.tile([A + 1, 128], F32, tag="ptr", name="pt_enc")
                nc.tensor.transpose(pt, at, identity)
                nc.any.tensor_copy(out=act_fm[:, t * 128:(t + 1) * 128], in_=pt)
            lnb = LNBatch()
            for t in range(c_tiles):
                pu = pp_lin.tile([128, D], F32, tag="plin", name="pu")
                nc.tensor.matmul(pu, act_fm[:, t * 128:(t + 1) * 128],
                                 w_ae_t, start=True, stop=True)
                xg = lnp_big.tile([128, D], F32, tag="xg", name="xg")
                nc.scalar.activation(out=xg, in_=pu, func=AF.Gelu)
                lnb.add(xg, "ln0", (lambda t=t: x_res[:, t, :]))
                nc.sync.dma_start(out=obs_tm[:, t, :],
                                  in_=obs_flat[tok0 + t * 128: tok0 + (t + 1) * 128, :])
            lnb.flush()

            # ---- blocks ----
            for bi in range(n_blocks):
                x1fm = big_pool.tile([128, KC, CT], F32R, tag="x1fm",
                                     name=f"x1fm_{ci}_{bi}")
                x_new = None
                for sub in range(2):  # 0: self-attn, 1: cross-attn
                    wq_t = load_w(wq_d[bi, sub], f"wq_{ci}_{bi}_{sub}")
                    wk_t = load_w(wk_d[bi, sub], f"wk_{ci}_{bi}_{sub}")
                    wv_t = load_w(wv_d[bi, sub], f"wv_{ci}_{bi}_{sub}")
                    wpj_t = load_w(wp_d[bi, sub], f"wp_{ci}_{bi}_{sub}")
                    if sub == 1:
                        x_new = res_pool.tile([128, c_tiles, D], F32, tag="res",
                                              name=f"x_new_{ci}_{bi}")
                    lnb = LNBatch()

                    for g in range(n_groups):
                        gcols = slice(g * 256, (g + 1) * 256)
                        # -- feature-major input for this group --
                        xfg = grp_pool.tile([128, KC, 256], F32R, tag="xfg", name="xfg")
                        src_tm = x_res if sub == 0 else obs_tm
                        for tt in range(2):
                            transpose_tm_tile(src_tm[:, g * 2 + tt, :], xfg,
                                              slice(tt * 128, (tt + 1) * 128))
                        q_src = xfg

                        def kv(kc_i):
                            if sub == 0:
                                return xfg[:, kc_i, :]
                            return x1fm[:, kc_i, gcols]

                        k_g = grp_pool.tile([128, KC, 256], ATT_DT, tag="kg", name="k_g")
                        for mc in range(KC):
                            pk = pp_lin.tile([128, 256], F32, tag="plin", name="pk")
                            for kc_i in range(KC):
                                nc.tensor.matmul(
                                    pk, wk_t[:, kc_i, mc * 128:(mc + 1) * 128],
                                    kv(kc_i),
                                    start=(kc_i == 0), stop=(kc_i == KC - 1))
                            if use_qk_bias:
                                nc.scalar.activation(out=k_g[:, mc, :], in_=pk,
                                                     func=AF.Identity,
                                                     bias=bk_t[bi][sub][:, mc:mc + 1],
                                                     scale=1.0)
                            else:
                                nc.any.tensor_copy(out=k_g[:, mc, :], in_=pk)

                        if att_bf16:
                            # v as full 128-token tiles (half the matmuls, full-lane
                            # evac); odd-seq rows DMA-shifted down to partitions 0:64
                            # so the AV contraction stays in base-0 geometry.
                            v_g2 = grp_pool.tile([128, 2, D], BF16, tag="vg", name="v_g2")
                            for tt in range(2):
                                pv = pp_lin.tile([128, D], F32, tag="plin", name="pv")
                                for kc_i in range(KC):
                                    nc.tensor.matmul(
                                        pv, kv(kc_i)[:, tt * 128:(tt + 1) * 128],
                                        wv_t[:, kc_i, :],
                                        start=(kc_i == 0), stop=(kc_i == KC - 1))
                                nc.any.tensor_copy(out=v_g2[:, tt, :], in_=pv)
                            v_sh = grp_pool.tile([64, 2, D], BF16, tag="vgs", name="v_sh")
                            nc.sync.dma_start(out=v_sh, in_=v_g2[64:128, :, :])

                            def vsl_of(sj, hp):
                                if sj % 2 == 0:
                                    return v_g2[0:64, sj // 2, hp * 128:(hp + 1) * 128]
                                return v_sh[:, sj // 2, hp * 128:(hp + 1) * 128]
                        else:
                            v_g = grp_pool.tile([64, 4, D], ATT_DT, tag="vg", name="v_g")
                            for sj in range(4):
                                pv = pp_lin.tile([64, D], F32, tag="plin", name="pv")
                                for kc_i in range(KC):
                                    nc.tensor.matmul(
                                        pv, kv(kc_i)[:, sj * 64:(sj + 1) * 64],
                                        wv_t[:, kc_i, :],
                                        start=(kc_i == 0), stop=(kc_i == KC - 1))
                                nc.any.tensor_copy(out=v_g[:, sj, :], in_=pv)

                            def vsl_of(sj, hp):
                                return v_g[:, sj, hp * 128:(hp + 1) * 128]

                        # -- attention (q built per head pair) --
                        y_g = grp_pool.tile([128, HP, 256], F32R, tag="yg", name="y_g")
                        for hp in range(HP):
                            pq = pp_lin.tile([128, 256], F32, tag="plin", name="pq")
                            for kc_i in range(KC):
                                nc.tensor.matmul(
                                    pq, wq_t[:, kc_i, hp * 128:(hp + 1) * 128],
                                    q_src[:, kc_i, :],
                                    start=(kc_i == 0), stop=(kc_i == KC - 1))
                            qb = qbd_tiles[(g * HP + hp) % 4]
                            if use_qk_bias:
                                nc.scalar.activation(
                                    out=qb[0:64, :, 0:64],
                                    in_=pq[0:64, :].rearrange("p (j f) -> p j f", f=64),
                                    func=AF.Identity, bias=bq_t[bi][sub][0:64, hp:hp + 1],
                                    scale=1.0)
                                nc.scalar.activation(
                                    out=qb[64:128, :, 64:128],
                                    in_=pq[64:128, :].rearrange("p (j f) -> p j f", f=64),
                                    func=AF.Identity, bias=bq_t[bi][sub][64:128, hp:hp + 1],
                                    scale=1.0)
                            else:
                                nc.any.tensor_copy(
                                    out=qb[0:64, :, 0:64],
                                    in_=pq[0:64, :].rearrange("p (j f) -> p j f", f=64))
                                nc.any.tensor_copy(
                                    out=qb[64:128, :, 64:128],
                                    in_=pq[64:128, :].rearrange("p (j f) -> p j f", f=64))
                            pa = pp_att.tile([128, 256], F32, tag="pa", name="pa")
                            for sj in range(4):
                                nc.tensor.matmul(pa[:, sj * 64:(sj + 1) * 64],
                                                 qb[:, sj, :],
                                                 k_g[:, hp, sj * 64:(sj + 1) * 64],
                                                 start=True, stop=True)
                            nc.vector.tensor_add(pa, pa, mask)
                            att_e = att_pool.tile([128, 256], F32, tag="atte", name="att_e")
                            nc.scalar.activation(out=att_e, in_=pa, func=AF.Exp)
                            den = sm_pool.tile([128, 4], F32, name="den")
                            nc.vector.reduce_sum(
                                out=den, in_=att_e.rearrange("p (j f) -> p j f", f=64),
                                axis=mybir.AxisListType.X)
                            rden = sm_pool.tile([128, 4], F32, name="rden")
                            nc.vector.reciprocal(out=rden, in_=den)
                            att_n = att_pool.tile([128, 256], ATT_DT, tag="attn", name="att_n")
                            for sj in range(4):
                                nc.vector.tensor_scalar_mul(
                                    att_n[:, sj * 64:(sj + 1) * 64],
                                    att_e[:, sj * 64:(sj + 1) * 64],
                                    rden[:, sj:sj + 1])
                            ptp = pp_tr.tile([64, 512], ATT_DT, tag="ptr", name="ptp")
                            for sj in range(4):
                                nc.tensor.transpose(ptp[:, sj * 128:(sj + 1) * 128],
                                                    att_n[:, sj * 64:(sj + 1) * 64],
                                                    identity_a)
                            attT = att_pool.tile([64, 512], ATT_DT, tag="attT", name="attT")
                            nc.any.tensor_copy(out=attT, in_=ptp)
                            py = pp_y.tile([128, 4, 128], F32, tag="py", name="py")
                            for sj in range(4):
                                nc.tensor.matmul(py[:, sj, :], vsl_of(sj, hp),
                                                 attT[:, sj * 128:(sj + 1) * 128],
                                                 start=True, stop=True)
                            nc.any.tensor_copy(out=y_g[0:64, hp, :],
                                               in_=py[0:64, :, 0:64])
                            nc.any.tensor_copy(out=y_g[64:128, hp, :],
                                               in_=py[64:128, :, 64:128])

                        # -- projection + residual + LN --
                        for tt in range(2):
                            t = g * 2 + tt
                            po = pp_lin.tile([128, D], F32, tag="plin", name="po")
                            for hp in range(HP):
                                nc.tensor.matmul(po, y_g[:, hp, tt * 128:(tt + 1) * 128],
                                                 wpj_t[:, hp, :],
                                                 start=(hp == 0), stop=(hp == HP - 1))
                            xpre = lnp_big.tile([128, D], F32, tag="xpre", name="xpre")
                            res_ap = x_res[:, t, :] if sub == 0 else obs_tm[:, t, :]
                            nc.vector.tensor_add(xpre, po, res_ap)
                            if use_bp:
                                nc.vector.tensor_add(xpre, xpre, bp_bc[bi][sub])
                            if sub == 0:
                                def mk_x1t():
                                    return ln_pool.tile([128, D], F32, tag="x1t",
                                                        name="x1t")

                                def post_x1(tgt, t=t):
                                    transpose_tm_tile(tgt, x1fm,
                                                      slice(t * 128, (t + 1) * 128))

                                lnb.add(xpre, (bi, 0), mk_x1t, post_x1)
                            else:
                                lnb.add(xpre, (bi, 1),
                                        (lambda t=t, xn=x_new: xn[:, t, :]))

                    lnb.flush()

                # ---- MLP sublayer ----
                w1_t = load_w(w1_d[bi], f"w1_{ci}_{bi}")
                w2_t = load_w(w2_d[bi], f"w2_{ci}_{bi}")
                x_res2 = x_new  # LN output of cross-attn sublayer
                x_out = res_pool.tile([128, c_tiles, D], F32, tag="res",
                                      name=f"x_out_{ci}_{bi}")
                lnb = LNBatch()
                for g in range(n_groups):
                    xfg2 = grp_pool.tile([128, KC, 256], F32R, tag="xfg", name="xfg2")
                    for tt in range(2):
                        transpose_tm_tile(x_res2[:, g * 2 + tt, :], xfg2,
                                          slice(tt * 128, (tt + 1) * 128))
                    m1 = grp_pool.tile([128, KC, 256], F32R, tag="m1", name="m1")
                    for mc in range(KC):
                        pm = pp_lin.tile([128, 256], F32, tag="plin", name="pm")
                        for kc_i in range(KC):
                            nc.tensor.matmul(pm, w1_t[:, kc_i, mc * 128:(mc + 1) * 128],
                                             xfg2[:, kc_i, :],
                                             start=(kc_i == 0), stop=(kc_i == KC - 1))
                        nc.scalar.activation(out=m1[:, mc, :], in_=pm, func=AF.Gelu,
                                             bias=b1_t[bi][:, mc:mc + 1], scale=1.0)
                    for tt in range(2):
                        t = g * 2 + tt
                        pm2 = pp_lin.tile([128, D], F32, tag="plin", name="pm2")
                        for kc_i in range(KC):
                            nc.tensor.matmul(pm2, m1[:, kc_i, tt * 128:(tt + 1) * 128],
                                             w2_t[:, kc_i, :],
                                             start=(kc_i == 0), stop=(kc_i == KC - 1))
                        xpre2 = lnp_big.tile([128, D], F32, tag="xpre", name="xpre2")
                        nc.vector.tensor_add(xpre2, pm2, x_res2[:, t, :])
                        if use_b2:
                            nc.vector.tensor_add(xpre2, xpre2, b2_bc[bi])
                        lnb.add(xpre2, (bi, 2),
                                (lambda t=t, xo=x_out: xo[:, t, :]))
                lnb.flush()
                x_res = x_out

            # ---- head ----
            wh1_t = load_w(wh1_d, f"wh1_{ci}")
            lnb = LNBatch()
            for g in range(n_groups):
                xfh = grp_pool.tile([128, KC, 256], F32R, tag="xfg", name="xfh")
                for tt in range(2):
                    transpose_tm_tile(x_res[:, g * 2 + tt, :], xfh,
                                      slice(tt * 128, (tt + 1) * 128))
                for tt in range(2):
                    t = g * 2 + tt
                    ph = pp_lin.tile([128, D], F32, tag="plin", name="ph")
                    for kc_i in range(KC):
                        nc.tensor.matmul(ph, xfh[:, kc_i, tt * 128:(tt + 1) * 128],
                                         wh1_t[:, kc_i, :],
                                         start=(kc_i == 0), stop=(kc_i == KC - 1))
                    hg = lnp_big.tile([128, D], F32, tag="xg", name="hg")
                    if use_bh1:
                        hpr = ln_pool.tile([128, D], F32, tag="xpre", name="hpr")
                        nc.vector.tensor_add(hpr, ph, bh1_bc)
                        nc.scalar.activation(out=hg, in_=hpr, func=AF.Gelu)
                    else:
                        nc.scalar.activation(out=hg, in_=ph, func=AF.Gelu)
                    def mk_hln():
                        return ln_pool.tile([128, D], F32, tag="x1t", name="hln")

                    def post_head(hln, t=t):
                        hfm = grp_pool.tile([128, KC, 128], F32R, tag="hfm",
                                            name="hfm")
                        transpose_tm_tile(hln, hfm, slice(0, 128))
                        pl = pp_att.tile([128, A], F32, tag="pa", name="pl")
                        for kc_i in range(KC):
                            nc.tensor.matmul(pl, hfm[:, kc_i, :], wh2_t[:, kc_i, :],
                                             start=(kc_i == 0), stop=(kc_i == KC - 1))
                        lt = ln_pool.tile([128, A], F32, tag="lt", name="lt")
                        if use_bh2:
                            nc.vector.tensor_add(lt, pl, bh2_bc)
                        else:
                            nc.any.tensor_copy(out=lt, in_=pl)
                        nc.sync.dma_start(
                            out=out_flat[tok0 + t * 128: tok0 + (t + 1) * 128, :],
                            in_=lt)

                    lnb.add(hg, "lnh", mk_hln, post_head)
            lnb.flush()

        for _p in reversed(_pools):
            _p.release()

    nc.compile()
    return nc


def make_mask():
    m = np.zeros((128, 4 * N), np.float32)
    qt = np.arange(64)
    base = np.where(qt[:, None] >= np.arange(64)[None, :], 0.0, -1e30).astype(np.float32)
    for half in range(2):
        for j in range(4):
            m[half * 64:(half + 1) * 64, j * 64:(j + 1) * 64] = base
    return m


def prepare_host_inputs(inputs, n_blocks=NB):
    """Fold scales/biases host-side; returns dict of per-core-shared arrays."""
    f = {k: np.asarray(v, dtype=np.float32) for k, v in inputs.items()}
    out = dict(f)
    # fold 1/sqrt(HD) into q projection
    out["blk_wq"] = f["blk_wq"] / np.float32(np.sqrt(HD))
    out["blk_bq"] = f["blk_bq"] / np.float32(np.sqrt(HD))
    # fold v bias through the projection: softmax rows sum to 1
    bp_eff = np.einsum("isd,isdm->ism", f["blk_bv"], f["blk_wp"]) + f["blk_bp"]
    out["blk_bp"] = bp_eff.astype(np.float32)
    out["const_mask"] = make_mask()
    out["const_identity"] = np.eye(128, dtype=np.float32)
    return out


_PROGRAM_CACHE = {}
ATT_BF16 = True
TRACE = False          # set True (e.g. from test.py) to capture an NTFF profile
LAST_RESULT = None     # BassKernelResults of the most recent kernel() call


def kernel(**inputs):
    host = prepare_host_inputs(inputs)
    flags = dict(
        att_bf16=ATT_BF16,
        use_qk_bias=bool(np.any(host["blk_bq"]) or np.any(host["blk_bk"])),
        use_bp=bool(np.any(host["blk_bp"])),
        use_b2=bool(np.any(host["blk_b2"])),
        use_bh1=bool(np.any(host["bh1"])),
        use_bh2=bool(np.any(host["bh2"])),
        ln_affine=bool(
            np.any(host["blk_ln_b"]) or np.any(host["blk_ln_g"] != 1.0)
            or np.any(host["ln0_b"]) or np.any(host["ln0_g"] != 1.0)
            or np.any(host["lnh_b"]) or np.any(host["lnh_g"] != 1.0)),
    )
    key = tuple(sorted(flags.items()))
    if key not in _PROGRAM_CACHE:
        _PROGRAM_CACHE[key] = build_program(**flags)
    nc = _PROGRAM_CACHE[key]

    shared_names = ["w_ae", "blk_wq", "blk_wk", "blk_wv", "blk_wp", "blk_bq",
                    "blk_bk", "blk_bp", "blk_w1", "blk_b1", "blk_w2", "blk_b2",
                    "wh1", "bh1", "wh2", "bh2", "blk_ln_g", "blk_ln_b",
                    "ln0_g", "ln0_b", "lnh_g", "lnh_b",
                    "const_mask", "const_identity"]
    in_maps = []
    for c in range(N_CORES):
        s0, s1 = c * SEQS_PER_CORE, (c + 1) * SEQS_PER_CORE
        m = {name: host[name] for name in shared_names}
        m["action"] = host["action"][s0:s1]
        m["obs_rep"] = host["obs_rep"][s0:s1]
        in_maps.append(m)

    global LAST_RESULT
    res = run_bass_kernel_spmd(nc, in_maps, list(range(N_CORES)), trace=TRACE)
    LAST_RESULT = res
    return np.concatenate([r["logit"] for r in res.results], axis=0)

